# revision 16
# baseline (speedup 1.0000x reference)
"""Upfirdn2d-style blur kernel for Trainium2 (Bass/Tile), 8-core SPMD.

Computes: zero-insertion 2x upsample + pad(2,1,2,1) + depthwise 4x4 FIR
  filter outer([1,3,3,1],[1,3,3,1])/64 * 4  (separable, symmetric)
on x of shape (16, 512, 32, 32) f32 -> (16, 512, 64, 64) f32.

Polyphase separable decomposition (verified vs reference, ~1e-7 abs err):
  vertical  : t[2r]   = (3*x[r] + x[r-1])/16 ; t[2r+1] = (3*x[r] + x[r+1])/16
  horizontal: o[2c]   = 3*t[c] + t[c-1]      ; o[2c+1] = 3*t[c] + t[c+1]
(out-of-range x/t taps are zero)

Sharding: pure data parallel over the 8192 independent images (batch*channel,
conv is depthwise) -> 1024 images per core, no cross-core communication.

Per-core layout: 32x32 images on SBUF partitions, 2 images per partition per
iteration (4 iterations x 256 images). Each 2-tap polyphase combine is ONE
fused DVE instruction (scalar_tensor_tensor: out = (in0 * scalar) + in1).

walrus here accepts only ONE sync-wait command per instruction, so the
program is structured so every instruction needs at most one new semaphore
observation:
  - exactly 8 DMAs total (4 in + 4 out) = the 8 DMAHW sem lanes, so no
    lane-reuse waits;
  - input DMAs use bufs=N_ITERS (no slot reuse -> no WAR/WAW waits);
  - all compute on DVE so compute deps fold into the single DVE sem;
  - a 1-element "wait absorber" op between the vertical and horizontal
    passes so the first o-writer carries only the DMA-out WAR wait.
"""

import numpy as np

import concourse.bass as bass
import concourse.mybir as mybir
import concourse.tile as tile
from concourse.bass_utils import run_bass_kernel_spmd

N_CORES = 8
B, C, H, W = 16, 512, 32, 32
IMGS = B * C                  # 8192 independent images
PER_CORE = IMGS // N_CORES    # 1024
P = 128                       # SBUF partitions
SUB = 2                       # images per partition per iteration
N_ITERS = PER_CORE // (P * SUB)   # 4
IMG = H * W                   # 1024 elems per input image
OIMG = 4 * IMG                # 4096 elems per output image

F32 = mybir.dt.float32
A = mybir.AluOpType


def _split_multi_waits(nc: bass.Bass) -> None:
    """walrus rejects >1 sync-wait per instruction; hoist extras onto NoOps.

    A NoOp on the same engine queue immediately before the instruction
    executes its wait first, so splitting the AND-list of waits across a
    NoOp chain is semantically identical.
    """
    for fn in nc.m.functions:
        for bb in fn.blocks:
            insts = bb.instructions
            i = 0
            while i < len(insts):
                inst = insts[i]
                si = inst.sync_info
                if si is not None and len(si.on_wait) > 1:
                    waits = list(si.on_wait)
                    for j, w in enumerate(waits[:-1]):
                        nop = mybir.InstNoOp(
                            name=nc.get_next_instruction_name(),
                            text_hint=f"wait_split_{j}")
                        nop.engine = inst.engine
                        nop.sync_info = mybir.SyncInfo(
                            on_wait=[w], on_update=[])
                        insts.insert(i, nop)
                        i += 1
                    inst.sync_info = mybir.SyncInfo(
                        on_wait=[waits[-1]], on_update=list(si.on_update))
                i += 1


def build_nc(split_waits: bool = True, repeat: int = 1,
             v_eng: str = "vector", sc_eng: str = "scalar",
             in_q: str = "scalar", out_q: str = "sync") -> bass.Bass:
    """Engine map (defaults): input DMA on ACT HWDGE queue, output DMA on SP
    queue (so outputs never block input prefetch), vertical STTs on GPSIMD,
    horizontal STTs on DVE, scalings/boundary taps on ACT. Multi-wait
    instructions are legalized by _split_multi_waits."""
    nc = bass.Bass()
    x = nc.dram_tensor("x", (PER_CORE, IMG), F32, kind="ExternalInput")
    out = nc.dram_tensor("out", (PER_CORE, OIMG), F32, kind="ExternalOutput")
    ev = getattr(nc, v_eng)      # vertical-pass STT engine
    sc = getattr(nc, sc_eng)     # pre-scale + boundary-tap engine
    in_dma = getattr(nc, in_q)
    out_dma = getattr(nc, out_q)

    with tile.TileContext(nc) as tc:
        with (
            tc.tile_pool(name="pin", bufs=N_ITERS) as pin,
            tc.tile_pool(name="pxq", bufs=2) as pxq,
            tc.tile_pool(name="pt", bufs=2) as pt,
            tc.tile_pool(name="po", bufs=2) as po,
        ):
            for it in range(repeat * N_ITERS):
                i = it % N_ITERS
                base = i * P * SUB

                xin = pin.tile([P, SUB * IMG], F32, tag="xin")
                # partition p holds images base+p and base+P+p
                x_dram = bass.AP(x, base * IMG,
                                 [[IMG, P], [P * IMG, SUB], [1, IMG]])
                xin_v = xin[:].rearrange("p (s c) -> p s c", s=SUB)
                in_dma.dma_start(out=xin_v, in_=x_dram)

                xq = pxq.tile([P, SUB * IMG], F32, tag="xq")
                t = pt.tile([P, SUB * 2 * IMG], F32, tag="t")
                o = po.tile([P, SUB * OIMG], F32, tag="o")

                # xq = x/16 — second-tap operand for the vertical pass
                for h in range(SUB):
                    sc.mul(xq[:, h * IMG:(h + 1) * IMG],
                           xin[:, h * IMG:(h + 1) * IMG], 1.0 / 16.0)

                for h in range(SUB):
                    x3 = xin[:, h * IMG:(h + 1) * IMG].rearrange(
                        "p (r c) -> p r c", c=W)
                    q3 = xq[:, h * IMG:(h + 1) * IMG].rearrange(
                        "p (r c) -> p r c", c=W)
                    th = t[:, h * 2 * IMG:(h + 1) * 2 * IMG]
                    t3 = th.rearrange("p (r c) -> p r c", c=W)
                    # t[2r] = (x[r]*3/16) + x[r-1]/16, r=1..31
                    ev.scalar_tensor_tensor(
                        t3[:, 2::2, :], x3[:, 1:, :], 3.0 / 16.0,
                        q3[:, :31, :], A.mult, A.add)
                    # t[2r+1] = (x[r]*3/16) + x[r+1]/16, r=0..30
                    ev.scalar_tensor_tensor(
                        t3[:, 1:62:2, :], x3[:, :31, :], 3.0 / 16.0,
                        q3[:, 1:, :], A.mult, A.add)
                    # boundary rows {0,63} <- (3/16) * x rows {0,31}
                    t_bnd = bass.AP(th.tensor, th.offset,
                                    [th.ap[0], [63 * W, 2], [1, W]])
                    x_bnd = bass.AP(xin[:].tensor,
                                    xin[:].offset + h * IMG,
                                    [xin[:].ap[0], [31 * W, 2], [1, W]])
                    sc.mul(t_bnd, x_bnd, 3.0 / 16.0)

                for h in range(SUB):
                    th = t[:, h * 2 * IMG:(h + 1) * 2 * IMG]
                    oh = o[:, h * OIMG:(h + 1) * OIMG]
                    t3 = th.rearrange("p (r c) -> p r c", c=W)
                    o3 = oh.rearrange("p (r c) -> p r c", c=2 * W)
                    # o[2c] = (t[c]*3) + t[c-1], c=1..31
                    nc.vector.scalar_tensor_tensor(
                        o3[:, :, 2::2], t3[:, :, 1:], 3.0, t3[:, :, :31],
                        A.mult, A.add)
                    # o[2c+1] = (t[c]*3) + t[c+1], c=0..30
                    nc.vector.scalar_tensor_tensor(
                        o3[:, :, 1:62:2], t3[:, :, :31], 3.0, t3[:, :, 1:],
                        A.mult, A.add)
                    # boundary cols {0,63} <- 3 * t cols {0,31}
                    o_bnd = bass.AP(oh.tensor, oh.offset,
                                    [oh.ap[0], [2 * W, 2 * H], [63, 2]])
                    t_bnd2 = bass.AP(th.tensor, th.offset,
                                     [th.ap[0], [W, 2 * H], [31, 2]])
                    sc.mul(o_bnd, t_bnd2, 3.0)

                o_dram = bass.AP(out, base * OIMG,
                                 [[OIMG, P], [P * OIMG, SUB], [1, OIMG]])
                o_v = o[:].rearrange("p (s c) -> p s c", s=SUB)
                out_dma.dma_start(out=o_dram, in_=o_v)
    if split_waits:
        _split_multi_waits(nc)
    return nc


def kernel(x: np.ndarray) -> np.ndarray:
    x = np.ascontiguousarray(np.asarray(x), dtype=np.float32)
    assert x.shape == (B, C, H, W), x.shape
    flat = x.reshape(IMGS, IMG)
    in_maps = [
        {"x": flat[c * PER_CORE:(c + 1) * PER_CORE]} for c in range(N_CORES)
    ]
    nc = build_nc()
    res = run_bass_kernel_spmd(nc, in_maps, core_ids=list(range(N_CORES)))
    outs = [res.results[c]["out"] for c in range(N_CORES)]
    full = np.concatenate(outs, axis=0).reshape(B, C, 2 * H, 2 * W)
    return full


if __name__ == "__main__":
    rng = np.random.default_rng(0)
    xt = rng.standard_normal((B, C, H, W), dtype=np.float32)
    yt = kernel(xt)
    print("out", yt.shape, yt.dtype)
